# revision 4
# baseline (speedup 1.0000x reference)
"""Bass/Trainium2 kernel for nn_Attention_5265629905090 — v3.

Masked single-head attention with linear projections (all bf16 matmuls —
fp8 measured 3-6e-2 rel err on host sim, over the 2e-2 gate).

Sharding: 8 cores = 4 batches x 2 query-halves, fully independent.

Host precomputes qm = enc_q @ M (M = W_q^T W_k / sqrt(D)) and
v_aug = [enc_v @ W_v^T, ones]; device does only QK + PV matmuls and the
masked exp. Final division (num / rowsum) on host.

v3 vs v2: QK and PV are interleaved at kc-tile granularity in ONE
software-pipelined loop (PV for tile j-LAG runs right after QK tile j),
so the PE never throttles to ACT's exp rate (v2's QK phases ran at
~690ns/tile against ACT instead of 432ns) and ACT/DVE never starve.
po psum banks are held across a whole chunk (4 live) and sized to a
full bank ([128,512] f32) so no two accumulation groups share a bank.
keep mask is bf16 (uint8 tensor_tensor measured ~50% slower on DVE).
"""

import numpy as np
import ml_dtypes

import concourse.bass as bass
import concourse.mybir as mybir
import concourse.tile as tile
from concourse.bass_utils import run_bass_kernel_spmd

BF16 = mybir.dt.bfloat16
F32 = mybir.dt.float32
U8 = mybir.dt.uint8

B, S, D = 4, 4096, 256
DE = D + 1           # 257: v columns + ones column (rowsum)
N_CORES = 8
SQ = S // 2          # query rows per core
KT = S // 128        # kc tiles (32)
CH = SQ // 512       # qr chunks of 512 (4)
LAG = 4              # kc-tiles of skew between QK and PV streams
NP_BF16 = ml_dtypes.bfloat16


def _split_excess_waits(nc: bass.Bass, max_waits: int = 1):
    """Walrus in this image rejects instructions carrying more than one
    sem wait (TPB_CTRL) / more than two (compute). Hoist extras onto
    same-engine InstNoOps inserted just before the instruction (engine
    program order preserves the happens-before)."""
    ctr = 0
    for f in nc.m.functions:
        for bb in f.blocks:
            new_insts = []
            for inst in bb.instructions:
                max_waits = 1
                si = inst.sync_info
                waits = list(si.on_wait) if (si and si.on_wait) else []
                if len(waits) > max_waits:
                    extras = waits[:-max_waits]
                    for i in range(0, len(extras), max_waits):
                        ctr += 1
                        nop = mybir.InstNoOp(
                            name=f"waitsplit-{ctr}", ins=[], outs=[]
                        )
                        nop.engine = inst.engine
                        nop.sync_info = mybir.SyncInfo(
                            on_wait=extras[i:i + max_waits], on_update=[]
                        )
                        new_insts.append(nop)
                    si.on_wait = waits[-max_waits:]
                new_insts.append(inst)
            bb.instructions[:] = new_insts


def build_nc() -> bass.Bass:
    nc = bass.Bass("TRN2", target_bir_lowering=False, debug=False,
                   num_devices=N_CORES)

    qmT_d = nc.declare_dram_parameter("qmT", [D, SQ], BF16, isOutput=False)
    ekT_d = nc.declare_dram_parameter("ekT", [D, S], BF16, isOutput=False)
    # v_aug pre-tiled on host: [g][p][j][e] = v_aug[g*1024 + j*128 + p, e]
    vaT_d = nc.declare_dram_parameter("vaT", [4, 128, 8, DE], BF16,
                                      isOutput=False)
    # keep pre-tiled on host (uint8 {0,1}): [ch*4+g][p][a*512+f] =
    #   keep[q = ch*512+f, k = (g*8+a)*128+p]
    keepT_d = nc.declare_dram_parameter("keepT", [CH * 4, 128, 8 * 512],
                                        U8, isOutput=False)
    out_d = nc.declare_dram_parameter("out", [SQ, DE], F32, isOutput=True)

    with tile.TileContext(nc) as tc:
        with (
            tc.tile_pool(name="consts", bufs=1) as consts,
            tc.tile_pool(name="ptp", bufs=1) as pt_pool,
            tc.tile_pool(name="keep", bufs=6) as keep_pool,
            tc.tile_pool(name="outs", bufs=6) as out_pool,
            tc.tile_pool(name="ps", bufs=3, space="PSUM") as ps_pool,
            tc.tile_pool(name="po", bufs=5, space="PSUM") as po_pool,
        ):
            # ---- PE warm-up: dummy matmuls during the initial DMA wait
            # so HAM un-throttles (1.2 -> 2.4 GHz) before real work.
            # memset on vector (gpsimd engine-init serialized ~2.5us in v2).
            wsrc = consts.tile([128, 512], BF16, tag="wsrc", name="wsrc")
            nc.vector.memset(wsrc, 0.0)
            wps = ps_pool.tile([128, 512], F32, tag="ps", name="wps")
            for i in range(8):
                nc.tensor.matmul(wps, lhsT=wsrc[:, 0:128], rhs=wsrc,
                                 start=True, stop=True)

            # ---- input DMAs (issue order ~ arrival order; fine-grained
            # so the first QK/PV tiles have their data early) ----
            qmT_sb = [consts.tile([128, SQ], BF16, tag=f"qm{t}",
                                  name=f"qm{t}") for t in range(2)]
            ekT_sb = [[consts.tile([128, 1024], BF16, tag=f"ek{t}g{g}",
                                   name=f"ek{t}g{g}") for g in range(4)]
                      for t in range(2)]
            va_sb = [consts.tile([128, 8, DE], BF16, tag=f"va{g}",
                                 name=f"va{g}") for g in range(4)]
            kp_tiles = {}

            def dma_ek(g):
                for t in range(2):
                    nc.sync.dma_start(
                        out=ekT_sb[t][g],
                        in_=ekT_d[t * 128:(t + 1) * 128,
                                  g * 1024:(g + 1) * 1024])

            def dma_qm(ch):
                for t in range(2):
                    nc.sync.dma_start(
                        out=qmT_sb[t][:, ch * 512:(ch + 1) * 512],
                        in_=qmT_d[t * 128:(t + 1) * 128,
                                  ch * 512:(ch + 1) * 512])

            # keep/va ride the SECOND hwdge queue (Activation) so the
            # startup feed uses both queues; their issue slots interleave
            # with exps but cost only ~600ns each against 195ns/tile slack.
            def dma_keep(ch, g):
                kp = keep_pool.tile([128, 8 * 512], U8, tag="keep",
                                    name=f"kp{ch}{g}")
                nc.scalar.dma_start(out=kp, in_=keepT_d[ch * 4 + g])
                kp_tiles[(ch, g)] = kp

            def dma_va(g):
                nc.scalar.dma_start(out=va_sb[g], in_=vaT_d[g])

            # PE-gating pieces first on each queue: QK j=0 needs ekT g0 +
            # qmT ch0; the mul needs keep00; PV j=0 needs va0.
            dma_ek(0)
            dma_qm(0)
            dma_keep(0, 0)
            dma_va(0)
            dma_ek(1)
            dma_keep(0, 1)
            dma_ek(2)
            dma_qm(1)
            dma_va(1)
            dma_ek(3)
            dma_keep(0, 2)
            dma_qm(2)
            dma_qm(3)
            dma_keep(0, 3)
            dma_va(2)
            dma_va(3)

            # ---- fused, software-pipelined QK+PV loop ----
            # Global tile index u = ch*KT + j runs over all 128 kc-tiles;
            # PV for tile u-LAG is emitted right after QK tile u.
            def pt_gen(ch):
                return [pt_pool.tile([128, 8 * 512], BF16,
                                     tag=f"pt{ch % 2}{g}",
                                     name=f"pt{ch % 2}{g}") for g in range(4)]

            pts = {}
            pos = {}

            def qk_tile(ch, j):
                g, a = divmod(j, 8)
                if (ch, g) not in kp_tiles:
                    dma_keep(ch, g)       # fallback; prefetch below avoids
                if j == 0:
                    pts[ch] = pt_gen(ch)
                # prefetch next chunk's keep group g while consuming this
                # chunk's group g (arrives ~24 tiles early; pool bufs=6
                # keeps at most ~6 live)
                if j % 8 == 4 and ch + 1 < CH and (ch + 1, g) not in kp_tiles:
                    dma_keep(ch + 1, g)
                ps = ps_pool.tile([128, 512], F32, tag="ps")
                for t_d in range(2):
                    nc.tensor.matmul(
                        ps,
                        lhsT=ekT_sb[t_d][g][:, a * 128:(a + 1) * 128],
                        rhs=qmT_sb[t_d][:, ch * 512:(ch + 1) * 512],
                        start=(t_d == 0), stop=(t_d == 1),
                    )
                sl = pts[ch][g][:, a * 512:(a + 1) * 512]
                nc.scalar.activation(
                    out=sl, in_=ps, func=mybir.ActivationFunctionType.Exp)
                kp = kp_tiles[(ch, g)]
                nc.vector.tensor_mul(sl, sl, kp[:, a * 512:(a + 1) * 512])

            def pv_tile(ch, j):
                g, a = divmod(j, 8)
                if j == 0:
                    pos[ch] = [po_pool.tile([128, 512], F32, tag="po",
                                            name=f"po{ch}{t}")
                               for t in range(4)]
                for t_q in range(4):
                    nc.tensor.matmul(
                        pos[ch][t_q][:, 0:DE],
                        lhsT=pts[ch][g][:, a * 512 + t_q * 128:
                                        a * 512 + (t_q + 1) * 128],
                        rhs=va_sb[g][:, a, :],
                        start=(j == 0), stop=(j == KT - 1),
                    )
                if j == KT - 1:
                    # alternate DVE/ACT so the four copies overlap; 6
                    # out-pool bufs so none waits on a prior DMA completion
                    for t_q in range(4):
                        o_sb = out_pool.tile([128, DE], F32, tag="osb",
                                             name="o_sb")
                        row0 = ch * 512 + t_q * 128
                        if t_q % 2 == 0:
                            nc.vector.tensor_copy(o_sb, pos[ch][t_q][:, 0:DE])
                            nc.sync.dma_start(out=out_d[row0:row0 + 128, :],
                                              in_=o_sb)
                        else:
                            nc.scalar.copy(o_sb, pos[ch][t_q][:, 0:DE])
                            nc.scalar.dma_start(out=out_d[row0:row0 + 128, :],
                                                in_=o_sb)

            NT = CH * KT
            for u in range(NT + LAG):
                if u < NT:
                    qk_tile(u // KT, u % KT)
                v = u - LAG
                if v >= 0:
                    pv_tile(v // KT, v % KT)
    _split_excess_waits(nc)
    return nc


_NC_CACHE = None


def _get_nc():
    global _NC_CACHE
    if _NC_CACHE is None:
        _NC_CACHE = build_nc()
    return _NC_CACHE


def _prep_core_inputs(encodings_q, encodings_k, encodings_v, mask,
                      W_q, W_k, W_v):
    """Host-side shard prep: projections + transposed bf16 layouts."""
    scale = 1.0 / np.sqrt(np.float32(D))
    M = ((W_q.T.astype(np.float64) @ W_k.astype(np.float64)) * scale
         ).astype(np.float32)
    WvT = W_v.T.astype(np.float32)
    keep = (~mask).astype(np.uint8)       # [B, S(q), S(k)]

    in_maps = []
    for c in range(N_CORES):
        b, h = divmod(c, 2)
        qs = slice(h * SQ, (h + 1) * SQ)
        qm = encodings_q[b, qs, :] @ M                    # [SQ, D] f32
        v = encodings_v[b] @ WvT                          # [S, D] f32
        va = np.ones((S, DE), dtype=NP_BF16)
        va[:, :D] = v.astype(NP_BF16)
        vaT = np.ascontiguousarray(
            va.reshape(4, 8, 128, DE).transpose(0, 2, 1, 3))
        ks = keep[b, qs, :]                               # [q=2048, k=4096]
        keepT = np.ascontiguousarray(
            ks.reshape(CH, 512, 4, 8, 128).transpose(0, 2, 4, 3, 1)
            .reshape(CH * 4, 128, 8 * 512))
        in_maps.append({
            "qmT": np.ascontiguousarray(qm.T.astype(NP_BF16)),
            "ekT": np.ascontiguousarray(encodings_k[b].T.astype(NP_BF16)),
            "vaT": vaT,
            "keepT": keepT,
        })
    return in_maps


def kernel(encodings_q, encodings_k, encodings_v, mask, W_q, W_k, W_v,
           **run_kwargs):
    nc = _get_nc()
    in_maps = _prep_core_inputs(
        np.asarray(encodings_q, dtype=np.float32),
        np.asarray(encodings_k, dtype=np.float32),
        np.asarray(encodings_v, dtype=np.float32),
        np.asarray(mask).astype(bool),
        np.asarray(W_q, dtype=np.float32),
        np.asarray(W_k, dtype=np.float32),
        np.asarray(W_v, dtype=np.float32),
    )
    res = run_bass_kernel_spmd(nc, in_maps, list(range(N_CORES)), **run_kwargs)
    out = np.empty((B, S, D), dtype=np.float32)
    for c in range(N_CORES):
        b, h = divmod(c, 2)
        o = res.results[c]["out"]                         # [SQ, 257] f32
        out[b, h * SQ:(h + 1) * SQ, :] = o[:, :D] / o[:, D:DE]
    if run_kwargs.get("trace"):
        kernel.last_exec_time_ns = res.exec_time_ns
    return out


# revision 5
# speedup vs baseline: 1.0155x; 1.0155x over previous
"""Bass/Trainium2 kernel for nn_Attention_5265629905090 — v3.

Masked single-head attention with linear projections (all bf16 matmuls —
fp8 measured 3-6e-2 rel err on host sim, over the 2e-2 gate).

Sharding: 8 cores = 4 batches x 2 query-halves, fully independent.

Host precomputes qm = enc_q @ M (M = W_q^T W_k / sqrt(D)) and
v_aug = [enc_v @ W_v^T, ones]; device does only QK + PV matmuls and the
masked exp. Final division (num / rowsum) on host.

v3 vs v2: QK and PV are interleaved at kc-tile granularity in ONE
software-pipelined loop (PV for tile j-LAG runs right after QK tile j),
so the PE never throttles to ACT's exp rate (v2's QK phases ran at
~690ns/tile against ACT instead of 432ns) and ACT/DVE never starve.
po psum banks are held across a whole chunk (4 live) and sized to a
full bank ([128,512] f32) so no two accumulation groups share a bank.
keep mask is bf16 (uint8 tensor_tensor measured ~50% slower on DVE).
"""

import numpy as np
import ml_dtypes

import concourse.bass as bass
import concourse.mybir as mybir
import concourse.tile as tile
from concourse.bass_utils import run_bass_kernel_spmd

BF16 = mybir.dt.bfloat16
F32 = mybir.dt.float32
U8 = mybir.dt.uint8

B, S, D = 4, 4096, 256
DE = D + 1           # 257: v columns + ones column (rowsum)
N_CORES = 8
SQ = S // 2          # query rows per core
KT = S // 128        # kc tiles (32)
CH = SQ // 512       # qr chunks of 512 (4)
LAG = 4              # kc-tiles of skew between QK and PV streams
NP_BF16 = ml_dtypes.bfloat16


def _split_excess_waits(nc: bass.Bass, max_waits: int = 1):
    """Walrus in this image rejects instructions carrying more than one
    sem wait (TPB_CTRL) / more than two (compute). Hoist extras onto
    same-engine InstNoOps inserted just before the instruction (engine
    program order preserves the happens-before)."""
    ctr = 0
    for f in nc.m.functions:
        for bb in f.blocks:
            new_insts = []
            for inst in bb.instructions:
                max_waits = 1
                si = inst.sync_info
                waits = list(si.on_wait) if (si and si.on_wait) else []
                if len(waits) > max_waits:
                    extras = waits[:-max_waits]
                    for i in range(0, len(extras), max_waits):
                        ctr += 1
                        nop = mybir.InstNoOp(
                            name=f"waitsplit-{ctr}", ins=[], outs=[]
                        )
                        nop.engine = inst.engine
                        nop.sync_info = mybir.SyncInfo(
                            on_wait=extras[i:i + max_waits], on_update=[]
                        )
                        new_insts.append(nop)
                    si.on_wait = waits[-max_waits:]
                new_insts.append(inst)
            bb.instructions[:] = new_insts


def build_nc() -> bass.Bass:
    nc = bass.Bass("TRN2", target_bir_lowering=False, debug=False,
                   num_devices=N_CORES)

    qmT_d = nc.declare_dram_parameter("qmT", [D, SQ], BF16, isOutput=False)
    ekT_d = nc.declare_dram_parameter("ekT", [D, S], BF16, isOutput=False)
    # v_aug pre-tiled on host: [g][p][j][e] = v_aug[g*1024 + j*128 + p, e]
    vaT_d = nc.declare_dram_parameter("vaT", [4, 128, 8, DE], BF16,
                                      isOutput=False)
    # keep pre-tiled on host (uint8 {0,1}): [ch*4+g][p][a*512+f] =
    #   keep[q = ch*512+f, k = (g*8+a)*128+p]
    keepT_d = nc.declare_dram_parameter("keepT", [CH * 4, 128, 8 * 512],
                                        U8, isOutput=False)
    out_d = nc.declare_dram_parameter("out", [SQ, DE], F32, isOutput=True)

    with tile.TileContext(nc) as tc:
        with (
            tc.tile_pool(name="consts", bufs=1) as consts,
            tc.tile_pool(name="ptp", bufs=1) as pt_pool,
            tc.tile_pool(name="keep", bufs=6) as keep_pool,
            tc.tile_pool(name="outs", bufs=6) as out_pool,
            tc.tile_pool(name="ps", bufs=3, space="PSUM") as ps_pool,
            tc.tile_pool(name="po", bufs=5, space="PSUM") as po_pool,
        ):
            # ---- PE warm-up: dummy matmuls during the initial DMA wait
            # so HAM un-throttles (1.2 -> 2.4 GHz) before real work.
            # memset on vector (gpsimd engine-init serialized ~2.5us in v2).
            wsrc = consts.tile([128, 512], BF16, tag="wsrc", name="wsrc")
            nc.vector.memset(wsrc, 0.0)
            wps = ps_pool.tile([128, 512], F32, tag="ps", name="wps")
            for i in range(8):
                nc.tensor.matmul(wps, lhsT=wsrc[:, 0:128], rhs=wsrc,
                                 start=True, stop=True)

            # ---- input DMAs (issue order ~ arrival order; fine-grained
            # so the first QK/PV tiles have their data early) ----
            qmT_sb = [consts.tile([128, SQ], BF16, tag=f"qm{t}",
                                  name=f"qm{t}") for t in range(2)]
            ekT_sb = [[consts.tile([128, 1024], BF16, tag=f"ek{t}g{g}",
                                   name=f"ek{t}g{g}") for g in range(4)]
                      for t in range(2)]
            va_sb = [consts.tile([128, 8, DE], BF16, tag=f"va{g}",
                                 name=f"va{g}") for g in range(4)]
            kp_tiles = {}

            def dma_ek(g):
                for t in range(2):
                    nc.sync.dma_start(
                        out=ekT_sb[t][g],
                        in_=ekT_d[t * 128:(t + 1) * 128,
                                  g * 1024:(g + 1) * 1024])

            def dma_qm(ch):
                for t in range(2):
                    nc.sync.dma_start(
                        out=qmT_sb[t][:, ch * 512:(ch + 1) * 512],
                        in_=qmT_d[t * 128:(t + 1) * 128,
                                  ch * 512:(ch + 1) * 512])

            # Startup keep/va ride the SECOND hwdge queue (Activation) so
            # the initial feed uses both queues' bandwidth; steady-state
            # prefetches go back on sync so ACT's queue stays clear for
            # exps (ACT budget is ~22us/chunk against a 28us chunk).
            def dma_keep(ch, g, eng=None):
                kp = keep_pool.tile([128, 8 * 512], U8, tag="keep",
                                    name=f"kp{ch}{g}")
                (eng or nc.sync).dma_start(out=kp, in_=keepT_d[ch * 4 + g])
                kp_tiles[(ch, g)] = kp

            def dma_va(g):
                nc.scalar.dma_start(out=va_sb[g], in_=vaT_d[g])

            # PE-gating pieces first on each queue: QK j=0 needs ekT g0 +
            # qmT ch0; the mul needs keep00; PV j=0 needs va0.
            dma_ek(0)
            dma_qm(0)
            dma_keep(0, 0, nc.scalar)
            dma_va(0)
            dma_ek(1)
            dma_keep(0, 1, nc.scalar)
            dma_ek(2)
            dma_qm(1)
            dma_va(1)
            dma_ek(3)
            dma_keep(0, 2, nc.scalar)
            dma_qm(2)
            dma_qm(3)
            dma_keep(0, 3, nc.scalar)
            dma_va(2)
            dma_va(3)

            # ---- fused, software-pipelined QK+PV loop ----
            # Global tile index u = ch*KT + j runs over all 128 kc-tiles;
            # PV for tile u-LAG is emitted right after QK tile u.
            def pt_gen(ch):
                return [pt_pool.tile([128, 8 * 512], BF16,
                                     tag=f"pt{ch % 2}{g}",
                                     name=f"pt{ch % 2}{g}") for g in range(4)]

            pts = {}
            pos = {}

            def qk_tile(ch, j):
                g, a = divmod(j, 8)
                if (ch, g) not in kp_tiles:
                    dma_keep(ch, g)       # fallback; prefetch below avoids
                if j == 0:
                    pts[ch] = pt_gen(ch)
                # prefetch next chunk's keep group g while consuming this
                # chunk's group g (arrives ~24 tiles early; pool bufs=6
                # keeps at most ~6 live)
                if j % 8 == 4 and ch + 1 < CH and (ch + 1, g) not in kp_tiles:
                    dma_keep(ch + 1, g)
                ps = ps_pool.tile([128, 512], F32, tag="ps")
                for t_d in range(2):
                    nc.tensor.matmul(
                        ps,
                        lhsT=ekT_sb[t_d][g][:, a * 128:(a + 1) * 128],
                        rhs=qmT_sb[t_d][:, ch * 512:(ch + 1) * 512],
                        start=(t_d == 0), stop=(t_d == 1),
                    )
                sl = pts[ch][g][:, a * 512:(a + 1) * 512]
                nc.scalar.activation(
                    out=sl, in_=ps, func=mybir.ActivationFunctionType.Exp)
                kp = kp_tiles[(ch, g)]
                nc.vector.tensor_mul(sl, sl, kp[:, a * 512:(a + 1) * 512])

            def pv_tile(ch, j):
                g, a = divmod(j, 8)
                if j == 0:
                    pos[ch] = [po_pool.tile([128, 512], F32, tag="po",
                                            name=f"po{ch}{t}")
                               for t in range(4)]
                for t_q in range(4):
                    nc.tensor.matmul(
                        pos[ch][t_q][:, 0:DE],
                        lhsT=pts[ch][g][:, a * 512 + t_q * 128:
                                        a * 512 + (t_q + 1) * 128],
                        rhs=va_sb[g][:, a, :],
                        start=(j == 0), stop=(j == KT - 1),
                    )
                if j == KT - 1:
                    # alternate DVE/ACT so the four copies overlap; 6
                    # out-pool bufs so none waits on a prior DMA completion
                    for t_q in range(4):
                        o_sb = out_pool.tile([128, DE], F32, tag="osb",
                                             name="o_sb")
                        row0 = ch * 512 + t_q * 128
                        if t_q % 2 == 0:
                            nc.vector.tensor_copy(o_sb, pos[ch][t_q][:, 0:DE])
                            nc.sync.dma_start(out=out_d[row0:row0 + 128, :],
                                              in_=o_sb)
                        else:
                            nc.scalar.copy(o_sb, pos[ch][t_q][:, 0:DE])
                            nc.scalar.dma_start(out=out_d[row0:row0 + 128, :],
                                                in_=o_sb)

            NT = CH * KT
            for u in range(NT + LAG):
                if u < NT:
                    qk_tile(u // KT, u % KT)
                v = u - LAG
                if v >= 0:
                    pv_tile(v // KT, v % KT)
    _split_excess_waits(nc)
    return nc


_NC_CACHE = None


def _get_nc():
    global _NC_CACHE
    if _NC_CACHE is None:
        _NC_CACHE = build_nc()
    return _NC_CACHE


def _prep_core_inputs(encodings_q, encodings_k, encodings_v, mask,
                      W_q, W_k, W_v):
    """Host-side shard prep: projections + transposed bf16 layouts."""
    scale = 1.0 / np.sqrt(np.float32(D))
    M = ((W_q.T.astype(np.float64) @ W_k.astype(np.float64)) * scale
         ).astype(np.float32)
    WvT = W_v.T.astype(np.float32)
    keep = (~mask).astype(np.uint8)       # [B, S(q), S(k)]

    in_maps = []
    for c in range(N_CORES):
        b, h = divmod(c, 2)
        qs = slice(h * SQ, (h + 1) * SQ)
        qm = encodings_q[b, qs, :] @ M                    # [SQ, D] f32
        v = encodings_v[b] @ WvT                          # [S, D] f32
        va = np.ones((S, DE), dtype=NP_BF16)
        va[:, :D] = v.astype(NP_BF16)
        vaT = np.ascontiguousarray(
            va.reshape(4, 8, 128, DE).transpose(0, 2, 1, 3))
        ks = keep[b, qs, :]                               # [q=2048, k=4096]
        keepT = np.ascontiguousarray(
            ks.reshape(CH, 512, 4, 8, 128).transpose(0, 2, 4, 3, 1)
            .reshape(CH * 4, 128, 8 * 512))
        in_maps.append({
            "qmT": np.ascontiguousarray(qm.T.astype(NP_BF16)),
            "ekT": np.ascontiguousarray(encodings_k[b].T.astype(NP_BF16)),
            "vaT": vaT,
            "keepT": keepT,
        })
    return in_maps


def kernel(encodings_q, encodings_k, encodings_v, mask, W_q, W_k, W_v,
           **run_kwargs):
    nc = _get_nc()
    in_maps = _prep_core_inputs(
        np.asarray(encodings_q, dtype=np.float32),
        np.asarray(encodings_k, dtype=np.float32),
        np.asarray(encodings_v, dtype=np.float32),
        np.asarray(mask).astype(bool),
        np.asarray(W_q, dtype=np.float32),
        np.asarray(W_k, dtype=np.float32),
        np.asarray(W_v, dtype=np.float32),
    )
    res = run_bass_kernel_spmd(nc, in_maps, list(range(N_CORES)), **run_kwargs)
    out = np.empty((B, S, D), dtype=np.float32)
    for c in range(N_CORES):
        b, h = divmod(c, 2)
        o = res.results[c]["out"]                         # [SQ, 257] f32
        out[b, h * SQ:(h + 1) * SQ, :] = o[:, :D] / o[:, D:DE]
    if run_kwargs.get("trace"):
        kernel.last_exec_time_ns = res.exec_time_ns
    return out


# revision 6
# speedup vs baseline: 1.0948x; 1.0781x over previous
"""Bass/Trainium2 kernel for nn_Attention_5265629905090 — v3.

Masked single-head attention with linear projections (all bf16 matmuls —
fp8 measured 3-6e-2 rel err on host sim, over the 2e-2 gate).

Sharding: 8 cores = 4 batches x 2 query-halves, fully independent.

Host precomputes qm = enc_q @ M (M = W_q^T W_k / sqrt(D)) and
v_aug = [enc_v @ W_v^T, ones]; device does only QK + PV matmuls and the
masked exp. Final division (num / rowsum) on host.

v3 vs v2: QK and PV are interleaved at kc-tile granularity in ONE
software-pipelined loop (PV for tile j-LAG runs right after QK tile j),
so the PE never throttles to ACT's exp rate (v2's QK phases ran at
~690ns/tile against ACT instead of 432ns) and ACT/DVE never starve.
po psum banks are held across a whole chunk (4 live) and sized to a
full bank ([128,512] f32) so no two accumulation groups share a bank.
keep mask is bf16 (uint8 tensor_tensor measured ~50% slower on DVE).
"""

import numpy as np
import ml_dtypes

import concourse.bass as bass
import concourse.mybir as mybir
import concourse.tile as tile
from concourse.bass_utils import run_bass_kernel_spmd

BF16 = mybir.dt.bfloat16
F32 = mybir.dt.float32
U8 = mybir.dt.uint8

B, S, D = 4, 4096, 256
DE = D + 1           # 257: v columns + ones column (rowsum)
N_CORES = 8
SQ = S // 2          # query rows per core
KT = S // 128        # kc tiles (32)
CH = SQ // 512       # qr chunks of 512 (4)
LAG = 4              # kc-tiles of skew between QK and PV streams
NP_BF16 = ml_dtypes.bfloat16


def _split_excess_waits(nc: bass.Bass, max_waits: int = 1):
    """Walrus in this image rejects instructions carrying more than one
    sem wait (TPB_CTRL) / more than two (compute). Hoist extras onto
    same-engine InstNoOps inserted just before the instruction (engine
    program order preserves the happens-before)."""
    ctr = 0
    for f in nc.m.functions:
        for bb in f.blocks:
            new_insts = []
            for inst in bb.instructions:
                max_waits = 1
                si = inst.sync_info
                waits = list(si.on_wait) if (si and si.on_wait) else []
                if len(waits) > max_waits:
                    extras = waits[:-max_waits]
                    for i in range(0, len(extras), max_waits):
                        ctr += 1
                        nop = mybir.InstNoOp(
                            name=f"waitsplit-{ctr}", ins=[], outs=[]
                        )
                        nop.engine = inst.engine
                        nop.sync_info = mybir.SyncInfo(
                            on_wait=extras[i:i + max_waits], on_update=[]
                        )
                        new_insts.append(nop)
                    si.on_wait = waits[-max_waits:]
                new_insts.append(inst)
            bb.instructions[:] = new_insts


def build_nc() -> bass.Bass:
    nc = bass.Bass("TRN2", target_bir_lowering=False, debug=False,
                   num_devices=N_CORES)

    qmT_d = nc.declare_dram_parameter("qmT", [D, SQ], BF16, isOutput=False)
    ekT_d = nc.declare_dram_parameter("ekT", [D, S], BF16, isOutput=False)
    # v_aug pre-tiled on host: [g][p][j][e] = v_aug[g*1024 + j*128 + p, e]
    vaT_d = nc.declare_dram_parameter("vaT", [4, 128, 8, DE], BF16,
                                      isOutput=False)
    # keep pre-tiled on host (uint8 {0,1}): [ch*4+g][p][a*512+f] =
    #   keep[q = ch*512+f, k = (g*8+a)*128+p]
    keepT_d = nc.declare_dram_parameter("keepT", [CH * 4, 128, 8 * 512],
                                        U8, isOutput=False)
    out_d = nc.declare_dram_parameter("out", [SQ, DE], F32, isOutput=True)

    with tile.TileContext(nc) as tc:
        with (
            tc.tile_pool(name="consts", bufs=1) as consts,
            tc.tile_pool(name="ptp", bufs=1) as pt_pool,
            tc.tile_pool(name="keep", bufs=6) as keep_pool,
            tc.tile_pool(name="outs", bufs=6) as out_pool,
            tc.tile_pool(name="ps", bufs=3, space="PSUM") as ps_pool,
            tc.tile_pool(name="po", bufs=5, space="PSUM") as po_pool,
        ):
            # ---- PE warm-up: dummy matmuls during the initial DMA wait
            # so HAM un-throttles (1.2 -> 2.4 GHz) before real work.
            # memset on gpsimd: its queue reaches work earliest (vector's
            # preamble delayed the first warm MM to 8.8us in v3/v5).
            wsrc = consts.tile([128, 512], BF16, tag="wsrc", name="wsrc")
            nc.gpsimd.memset(wsrc, 0.0)
            wps = ps_pool.tile([128, 512], F32, tag="ps", name="wps")
            for i in range(8):
                nc.tensor.matmul(wps, lhsT=wsrc[:, 0:128], rhs=wsrc,
                                 start=True, stop=True)
            # ---- ACT warm-up: the Exp LUT loads lazily on first use
            # (ACT_TABLE_LOAD, 1.3us — measured blocking the first real
            # exp until 26us in v5). Trigger it now on a tiny self-
            # initialized tile, all on the scalar queue.
            wact = consts.tile([128, 8], F32, tag="wact", name="wact")
            nc.scalar.memzero(wact)
            nc.scalar.activation(out=wact, in_=wact,
                                 func=mybir.ActivationFunctionType.Exp)

            # ---- input DMAs (issue order ~ arrival order; fine-grained
            # so the first QK/PV tiles have their data early) ----
            qmT_sb = [consts.tile([128, SQ], BF16, tag=f"qm{t}",
                                  name=f"qm{t}") for t in range(2)]
            ekT_sb = [[consts.tile([128, 1024], BF16, tag=f"ek{t}g{g}",
                                   name=f"ek{t}g{g}") for g in range(4)]
                      for t in range(2)]
            va_sb = [consts.tile([128, 8, DE], BF16, tag=f"va{g}",
                                 name=f"va{g}") for g in range(4)]
            kp_tiles = {}

            def dma_ek(g):
                for t in range(2):
                    nc.sync.dma_start(
                        out=ekT_sb[t][g],
                        in_=ekT_d[t * 128:(t + 1) * 128,
                                  g * 1024:(g + 1) * 1024])

            # All input DMAs on the sync queue, in consumption order —
            # measured BW is ~390GB/s burst, so ordering (not bandwidth)
            # is what gates the start. DMAs on the scalar queue stall the
            # exp stream (head-of-line, measured v4/v5), so none go there
            # until the final epilogue.
            def dma_keep(ch, g):
                kp = keep_pool.tile([128, 8 * 512], U8, tag="keep",
                                    name=f"kp{ch}{g}")
                nc.sync.dma_start(out=kp, in_=keepT_d[ch * 4 + g])
                kp_tiles[(ch, g)] = kp

            def dma_va(g):
                nc.sync.dma_start(out=va_sb[g], in_=vaT_d[g])

            dma_ek(0)
            for t in range(2):
                nc.sync.dma_start(out=qmT_sb[t],
                                  in_=qmT_d[t * 128:(t + 1) * 128, :])
            dma_keep(0, 0)
            dma_va(0)
            dma_ek(1)
            dma_keep(0, 1)
            dma_ek(2)
            dma_va(1)
            dma_ek(3)
            dma_keep(0, 2)
            dma_keep(0, 3)
            dma_va(2)
            dma_va(3)

            # ---- fused, software-pipelined QK+PV loop ----
            # Global tile index u = ch*KT + j runs over all 128 kc-tiles;
            # PV for tile u-LAG is emitted right after QK tile u.
            def pt_gen(ch):
                return [pt_pool.tile([128, 8 * 512], BF16,
                                     tag=f"pt{ch % 2}{g}",
                                     name=f"pt{ch % 2}{g}") for g in range(4)]

            pts = {}
            pos = {}

            def qk_tile(ch, j):
                g, a = divmod(j, 8)
                if (ch, g) not in kp_tiles:
                    dma_keep(ch, g)       # fallback; prefetch below avoids
                if j == 0:
                    pts[ch] = pt_gen(ch)
                # prefetch next chunk's keep group g while consuming this
                # chunk's group g (arrives ~24 tiles early; pool bufs=6
                # keeps at most ~6 live)
                if j % 8 == 4 and ch + 1 < CH and (ch + 1, g) not in kp_tiles:
                    dma_keep(ch + 1, g)
                ps = ps_pool.tile([128, 512], F32, tag="ps")
                for t_d in range(2):
                    nc.tensor.matmul(
                        ps,
                        lhsT=ekT_sb[t_d][g][:, a * 128:(a + 1) * 128],
                        rhs=qmT_sb[t_d][:, ch * 512:(ch + 1) * 512],
                        start=(t_d == 0), stop=(t_d == 1),
                    )
                sl = pts[ch][g][:, a * 512:(a + 1) * 512]
                nc.scalar.activation(
                    out=sl, in_=ps, func=mybir.ActivationFunctionType.Exp)
                kp = kp_tiles[(ch, g)]
                nc.vector.tensor_mul(sl, sl, kp[:, a * 512:(a + 1) * 512])

            def pv_tile(ch, j):
                g, a = divmod(j, 8)
                if j == 0:
                    pos[ch] = [po_pool.tile([128, 512], F32, tag="po",
                                            name=f"po{ch}{t}")
                               for t in range(4)]
                for t_q in range(4):
                    nc.tensor.matmul(
                        pos[ch][t_q][:, 0:DE],
                        lhsT=pts[ch][g][:, a * 512 + t_q * 128:
                                        a * 512 + (t_q + 1) * 128],
                        rhs=va_sb[g][:, a, :],
                        start=(j == 0), stop=(j == KT - 1),
                    )
                if j == KT - 1:
                    # alternate DVE/ACT so the four copies overlap; 6
                    # out-pool bufs so none waits on a prior DMA completion
                    for t_q in range(4):
                        o_sb = out_pool.tile([128, DE], F32, tag="osb",
                                             name="o_sb")
                        row0 = ch * 512 + t_q * 128
                        if t_q % 2 == 0:
                            nc.vector.tensor_copy(o_sb, pos[ch][t_q][:, 0:DE])
                            nc.sync.dma_start(out=out_d[row0:row0 + 128, :],
                                              in_=o_sb)
                        else:
                            nc.scalar.copy(o_sb, pos[ch][t_q][:, 0:DE])
                            nc.scalar.dma_start(out=out_d[row0:row0 + 128, :],
                                                in_=o_sb)

            NT = CH * KT
            for u in range(NT + LAG):
                if u < NT:
                    qk_tile(u // KT, u % KT)
                v = u - LAG
                if v >= 0:
                    pv_tile(v // KT, v % KT)
    _split_excess_waits(nc)
    return nc


_NC_CACHE = None


def _get_nc():
    global _NC_CACHE
    if _NC_CACHE is None:
        _NC_CACHE = build_nc()
    return _NC_CACHE


def _prep_core_inputs(encodings_q, encodings_k, encodings_v, mask,
                      W_q, W_k, W_v):
    """Host-side shard prep: projections + transposed bf16 layouts."""
    scale = 1.0 / np.sqrt(np.float32(D))
    M = ((W_q.T.astype(np.float64) @ W_k.astype(np.float64)) * scale
         ).astype(np.float32)
    WvT = W_v.T.astype(np.float32)
    keep = (~mask).astype(np.uint8)       # [B, S(q), S(k)]

    in_maps = []
    for c in range(N_CORES):
        b, h = divmod(c, 2)
        qs = slice(h * SQ, (h + 1) * SQ)
        qm = encodings_q[b, qs, :] @ M                    # [SQ, D] f32
        v = encodings_v[b] @ WvT                          # [S, D] f32
        va = np.ones((S, DE), dtype=NP_BF16)
        va[:, :D] = v.astype(NP_BF16)
        vaT = np.ascontiguousarray(
            va.reshape(4, 8, 128, DE).transpose(0, 2, 1, 3))
        ks = keep[b, qs, :]                               # [q=2048, k=4096]
        keepT = np.ascontiguousarray(
            ks.reshape(CH, 512, 4, 8, 128).transpose(0, 2, 4, 3, 1)
            .reshape(CH * 4, 128, 8 * 512))
        in_maps.append({
            "qmT": np.ascontiguousarray(qm.T.astype(NP_BF16)),
            "ekT": np.ascontiguousarray(encodings_k[b].T.astype(NP_BF16)),
            "vaT": vaT,
            "keepT": keepT,
        })
    return in_maps


def kernel(encodings_q, encodings_k, encodings_v, mask, W_q, W_k, W_v,
           **run_kwargs):
    nc = _get_nc()
    in_maps = _prep_core_inputs(
        np.asarray(encodings_q, dtype=np.float32),
        np.asarray(encodings_k, dtype=np.float32),
        np.asarray(encodings_v, dtype=np.float32),
        np.asarray(mask).astype(bool),
        np.asarray(W_q, dtype=np.float32),
        np.asarray(W_k, dtype=np.float32),
        np.asarray(W_v, dtype=np.float32),
    )
    res = run_bass_kernel_spmd(nc, in_maps, list(range(N_CORES)), **run_kwargs)
    out = np.empty((B, S, D), dtype=np.float32)
    for c in range(N_CORES):
        b, h = divmod(c, 2)
        o = res.results[c]["out"]                         # [SQ, 257] f32
        out[b, h * SQ:(h + 1) * SQ, :] = o[:, :D] / o[:, D:DE]
    if run_kwargs.get("trace"):
        kernel.last_exec_time_ns = res.exec_time_ns
    return out
